# revision 11
# baseline (speedup 1.0000x reference)
# Trainium2 Bass kernel for AvaAttention (GQA attention + RoPE + additive mask)
# B=2, T=2048, HID=2048, NH=16, KVH=4, HD=128, fp32 in/out — 8 NeuronCores.
#
# Sharding: sequence-parallel. Core i (batch b=i//4, position p=i%4) owns
# q-blocks j = 4s+3-p of batch b, for slot s in 0..3. Projections are
# row-parallel (weights replicated, host-cast to bf16), K/V exchanged with
# an AllGather over each batch's 4 cores, attention + output projection
# stay local to the core's rows.
#
# v2 design (vs the earlier kernel):
#  - hidden_states arrives pre-transposed from the host (xT), removing the
#    on-device transpose phase entirely.
#  - Attention computes scores directly in [src, (head q)] orientation
#    (stationary = kT block, moving = qT), so the probability tiles come
#    out of exp already transposed for the PV matmul: no per-tile
#    probability transpose, no diag matmuls, no P^T copies on vector.
#  - Softmax denominator Z is accumulated with col-tiled M=1 matmuls
#    (4 concurrent positions at PSUM partitions 0/32/64/96), then one
#    masked-ones matmul sums the partials and broadcasts them across all
#    128 partitions in a single step; a vector reciprocal + one
#    tensor_tensor multiply normalizes ctx while casting to bf16.
#  - The causal mask is applied as a 0/1 multiply on the probability
#    tiles (cheap vector op) instead of -1e9 adds via identity matmuls.
#  - exp runs on the scalar engine in [128, 2048]/[128, 1024] PSUM
#    batches to amortize the per-instruction overhead.
#  - RoPE runs in bf16 on the vector engine (PSUM->bf16 staging copies on
#    the scalar engine) for 2x DVE throughput.

import sys

for _p in ("/opt/trn_rl_repo", "/opt/pypackages"):
    if _p not in sys.path:
        sys.path.insert(0, _p)

import numpy as np
import ml_dtypes

B, T, HID = 2, 2048, 2048
NH, KVH, HD = 16, 4, 128
P = 128
NC = 8
NBLK = T // P          # 16 q-blocks per batch
NSLOT = 4              # blocks per core
GPQ = NH // KVH        # 4 q-heads per kv group
HB = HID // P          # 16 contraction subtiles
NEG_THRESH = -1.0e8


def _c_of_j(j):
    # producer-permuted column-block index (involution)
    return 4 * (j // 4) + 3 - (j % 4)


def _mask_plan(attention_mask):
    """Classify the additive mask per (j, kb) 128x128 tile.

    Returns (E, P_list): E[s] is the uniform extent (in permuted blocks c)
    for slot s; P_list is the ordered list of (s, c) positions where a
    0/1 mask multiply is applied (positions shared by every core; tile
    *data* is per-core).
    """
    m = np.asarray(attention_mask).reshape(T, T)
    nonzero = np.zeros((NBLK, NBLK), dtype=bool)
    live = np.zeros((NBLK, NBLK), dtype=bool)   # not fully masked
    for j in range(NBLK):
        for kb in range(NBLK):
            tile = m[j * P:(j + 1) * P, kb * P:(kb + 1) * P]
            nonzero[j, kb] = bool(np.any(tile != 0.0))
            live[j, kb] = bool(np.any(tile > NEG_THRESH))
    E = []
    for s in range(NSLOT):
        cmax = 1
        for jj in range(4):
            j = 4 * s + jj
            idx = np.nonzero(live[j])[0]
            if len(idx):
                cmax = max(cmax, max(_c_of_j(int(kb)) for kb in idx) + 1)
        E.append(cmax)
    P_list = []
    for s in range(NSLOT):
        for c in range(E[s]):
            kb = _c_of_j(c)
            if any(nonzero[4 * s + jj, kb] for jj in range(4)):
                P_list.append((s, c))
    return E, P_list


def _batches(Es):
    """Split Es blocks into exp batches alternating 4 (tile A) / 2 (tile B)."""
    out = []
    start = 0
    use_a = True
    while start < Es:
        take = min(4 if use_a else 2, Es - start)
        out.append((start, take, use_a))
        start += take
        use_a = not use_a
    return out


def _build_program(E, P_list):
    import concourse.mybir as mybir
    import concourse.tile as tile
    from concourse import bacc
    from concourse.masks import make_identity
    from contextlib import ExitStack

    FP32 = mybir.dt.float32
    BF16 = mybir.dt.bfloat16
    Exp = mybir.ActivationFunctionType.Exp
    HALF = HD // 2

    nc = bacc.Bacc("TRN2", target_bir_lowering=False, num_devices=NC)

    xT_p = nc.declare_dram_parameter("xT", [HID, NSLOT * P], BF16, isOutput=False)
    wq_p = nc.declare_dram_parameter("wq", [HID, NH * HD], BF16, isOutput=False)
    wk_p = nc.declare_dram_parameter("wk", [HID, KVH * HD], BF16, isOutput=False)
    wv_p = nc.declare_dram_parameter("wv", [HID, KVH * HD], BF16, isOutput=False)
    wo_p = nc.declare_dram_parameter("wo", [HID, HID], BF16, isOutput=False)
    cosq_p = nc.declare_dram_parameter("cosq", [NSLOT * P, HD], BF16, isOutput=False)
    sinq_p = nc.declare_dram_parameter("sinq3", [NSLOT * P, HD], BF16, isOutput=False)
    cosk_p = nc.declare_dram_parameter("cosk", [NSLOT * P, HD], BF16, isOutput=False)
    sink_p = nc.declare_dram_parameter("sink3", [NSLOT * P, HD], BF16, isOutput=False)
    nmask = max(1, len(P_list))
    masks_p = nc.declare_dram_parameter("maskbin", [nmask, P, P], BF16,
                                        isOutput=False)
    out_p = nc.declare_dram_parameter("out", [NSLOT * P, HID], FP32, isOutput=True)

    KVW = KVH * HD  # 512
    ag_k_in = nc.dram_tensor("ag_k_in", [KVW, NSLOT * P], BF16)
    ag_k_out = nc.dram_tensor("ag_k_out", [4, KVW, NSLOT * P], BF16,
                              addr_space="Local")
    ag_v_in = nc.dram_tensor("ag_v_in", [NSLOT * P, KVW], BF16)
    ag_v_out = nc.dram_tensor("ag_v_out", [4, NSLOT * P, KVW], BF16,
                              addr_space="Local")
    groups = [[0, 1, 2, 3], [4, 5, 6, 7]]

    mask_idx = {sk: idx for idx, sk in enumerate(P_list)}

    with tile.TileContext(nc) as tc, ExitStack() as top:
        const = top.enter_context(tc.tile_pool(name="const", bufs=1))
        ident_bf = const.tile([P, P], BF16)
        make_identity(nc, ident_bf[:])
        ones1 = const.tile([P, 1], BF16)
        nc.vector.memset(ones1[:], 1.0)
        ones97 = const.tile([97, P], BF16)
        nc.vector.memset(ones97[:], 0.0)
        for r in (0, 32, 64, 96):
            nc.vector.memset(ones97[r:r + 1, :], 1.0)
        maskbin = const.tile([P, nmask, P], BF16)

        # long-lived attention inputs
        qT_pool = top.enter_context(tc.tile_pool(name="qT_pool", bufs=1))
        qT = qT_pool.tile([P, KVH, NSLOT, GPQ, P], BF16)   # [d, g, s, h, t]

        # ================= projection phase =================
        with tc.tile_pool(name="xw_pool", bufs=1) as xw_pool, \
             tc.tile_pool(name="qw", bufs=2) as qw_pool, \
             tc.tile_pool(name="stage", bufs=2) as stage, \
             tc.tile_pool(name="ropec", bufs=1) as ropec, \
             tc.tile_pool(name="pps", bufs=1, space="PSUM") as pps:
            xT = xw_pool.tile([P, HB, NSLOT * P], BF16)    # [h%128, hb, t]
            wk_sb = xw_pool.tile([P, HB, KVW], BF16)
            wv_sb = xw_pool.tile([P, HB, KVW], BF16)
            nc.sync.dma_start(xT[:], xT_p[:].rearrange("(hb p) t -> p hb t", p=P))
            nc.sync.dma_start(wk_sb[:], wk_p[:].rearrange("(hb p) n -> p hb n", p=P))
            nc.sync.dma_start(wv_sb[:], wv_p[:].rearrange("(hb p) n -> p hb n", p=P))
            cosq_t = ropec.tile([P, NSLOT, HD], BF16)
            sinq_t = ropec.tile([P, NSLOT, HD], BF16)
            cosk_t = ropec.tile([P, NSLOT, HD], BF16)
            sink_t = ropec.tile([P, NSLOT, HD], BF16)
            for ap, prm in ((cosk_t, cosk_p), (sink_t, sink_p),
                            (cosq_t, cosq_p), (sinq_t, sinq_p)):
                nc.sync.dma_start(ap[:], prm[:].rearrange("(s p) d -> p s d", p=P))
            nc.sync.dma_start(maskbin[:], masks_p[:].rearrange("n p d -> p n d"))
            # prefetch the first two Wq chunks
            wq_tiles = {}
            for hc in range(2):
                w = qw_pool.tile([P, HB, GPQ * HD], BF16, name="wq_sb", tag="wq")
                nc.sync.dma_start(
                    w[:], wq_p[:, hc * GPQ * HD:(hc + 1) * GPQ * HD]
                    .rearrange("(hb p) n -> p hb n", p=P))
                wq_tiles[hc] = w

            def rope(dst, src_sb, cos_t, sin_t, s, nh):
                src3 = src_sb[:].rearrange("p (h d) -> p h d", d=HD)
                cst = stage.tile([P, nh, HD], BF16, name="rope_c", tag="rope_c")
                nc.vector.tensor_tensor(
                    dst[:], src3,
                    cos_t[:, s, None, :].to_broadcast((P, nh, HD)),
                    mybir.AluOpType.mult)
                nc.vector.tensor_tensor(
                    cst[:], src3,
                    sin_t[:, s, None, :].to_broadcast((P, nh, HD)),
                    mybir.AluOpType.mult)
                nc.vector.tensor_tensor(dst[:, :, HALF:], dst[:, :, HALF:],
                                        cst[:, :, :HALF], mybir.AluOpType.add)
                nc.vector.tensor_tensor(dst[:, :, :HALF], dst[:, :, :HALF],
                                        cst[:, :, HALF:], mybir.AluOpType.add)

            # ---- K projection + rope + transpose + AllGather ----
            pk = [pps.tile([P, KVW], FP32, name=f"pk{s}", tag=f"pkv{s}")
                  for s in range(NSLOT)]
            for hb in range(HB):
                for s in range(NSLOT):
                    nc.tensor.matmul(pk[s][:], xT[:, hb, s * P:(s + 1) * P],
                                     wk_sb[:, hb, :],
                                     start=(hb == 0), stop=(hb == HB - 1))
            k_rope = []
            for s in range(NSLOT):
                kst = stage.tile([P, KVW], BF16, name=f"kst{s}", tag=f"kst{s % 2}")
                nc.scalar.copy(kst[:], pk[s][:])
                kr = xw_pool.tile([P, KVH, HD], BF16, name=f"k_rope{s}")
                rope(kr, kst, cosk_t, sink_t, s, KVH)
                k_rope.append(kr)
            contrib_k = xw_pool.tile([P, KVH, NSLOT * P], BF16, name="contrib_k")
            for g in range(KVH):
                pkt = pps.tile([P, NSLOT * P], BF16, name="pkt", tag=f"pkt{g % 2}")
                for s in range(NSLOT):
                    nc.tensor.transpose(pkt[:, s * P:(s + 1) * P],
                                        k_rope[s][:, g, :], ident_bf[:])
                nc.vector.tensor_copy(contrib_k[:, g, :], pkt[:])
            nc.sync.dma_start(
                ag_k_in[:].rearrange("(g d) t -> d g t", d=P), contrib_k[:])
            nc.gpsimd.collective_compute(
                "AllGather", mybir.AluOpType.bypass, replica_groups=groups,
                ins=[ag_k_in[:]], outs=[ag_k_out[:]])

            # ---- V projection + AllGather ----
            pv = [pps.tile([P, KVW], FP32, name=f"pv{s}", tag=f"pkv{s}")
                  for s in range(NSLOT)]
            for hb in range(HB):
                for s in range(NSLOT):
                    nc.tensor.matmul(pv[s][:], xT[:, hb, s * P:(s + 1) * P],
                                     wv_sb[:, hb, :],
                                     start=(hb == 0), stop=(hb == HB - 1))
            for s in range(NSLOT):
                vst = stage.tile([P, KVW], BF16, name=f"vst{s}", tag="vst")
                nc.scalar.copy(vst[:], pv[s][:])
                nc.sync.dma_start(ag_v_in[s * P:(s + 1) * P, :], vst[:])
            nc.gpsimd.collective_compute(
                "AllGather", mybir.AluOpType.bypass, replica_groups=groups,
                ins=[ag_v_in[:]], outs=[ag_v_out[:]])

            # ---- Q projection + rope + transpose to qT ----
            for hc in range(NH // GPQ):
                if hc in wq_tiles:
                    wq_sb = wq_tiles[hc]
                else:
                    wq_sb = qw_pool.tile([P, HB, GPQ * HD], BF16,
                                         name="wq_sb", tag="wq")
                    nc.sync.dma_start(
                        wq_sb[:], wq_p[:, hc * GPQ * HD:(hc + 1) * GPQ * HD]
                        .rearrange("(hb p) n -> p hb n", p=P))
                pq = [pps.tile([P, GPQ * HD], FP32, name=f"pq{s}", tag=f"pkv{s}")
                      for s in range(NSLOT)]
                for hb in range(HB):
                    for s in range(NSLOT):
                        nc.tensor.matmul(pq[s][:], xT[:, hb, s * P:(s + 1) * P],
                                         wq_sb[:, hb, :],
                                         start=(hb == 0), stop=(hb == HB - 1))
                for s in range(NSLOT):
                    qst = stage.tile([P, GPQ * HD], BF16, name="qst",
                                     tag=f"qst{s % 2}")
                    nc.scalar.copy(qst[:], pq[s][:])
                    qr = stage.tile([P, GPQ, HD], BF16, name="q_rope",
                                    tag=f"q_rope{s % 2}")
                    rope(qr, qst, cosq_t, sinq_t, s, GPQ)
                    pqt = pps.tile([P, GPQ * P], BF16, name="pqt",
                                   tag=f"pqt{s % 2}")
                    for h in range(GPQ):
                        nc.tensor.transpose(pqt[:, h * P:(h + 1) * P],
                                            qr[:, h, :], ident_bf[:])
                    nc.vector.tensor_copy(
                        qT[:, hc, s, :, :],
                        pqt[:].rearrange("p (h t) -> p h t", t=P))

        # ---- gather K/V from the AllGather outputs; preload wo ----
        # (pool created after the projection pools release their SBUF)
        kv_pool = top.enter_context(tc.tile_pool(name="kv_pool", bufs=1))
        # c = 4*s_src + pos is addressed as [s_src, pos]
        kT = kv_pool.tile([P, KVH, NSLOT, 4, P], BF16)     # [d, g, s_src, pos, t]
        v_all = kv_pool.tile([P, NBLK, KVW], BF16)         # [t%128, c, (g d)]
        ctxT = kv_pool.tile([P, NSLOT, NH, P], BF16)       # [d, s, hh, t]
        wo_sb = kv_pool.tile([P, HB, HID], BF16)
        for g in range(KVH):
            for pos in range(4):
                nc.sync.dma_start(
                    kT[:, g, :, pos, :],
                    ag_k_out[pos, g * P:(g + 1) * P, :]
                    .rearrange("d (s t) -> d s t", t=P))
        for s_src in range(NSLOT):
            for pos in range(4):
                nc.sync.dma_start(v_all[:, 4 * s_src + pos, :],
                                  ag_v_out[pos, s_src * P:(s_src + 1) * P, :])
        nc.sync.dma_start(wo_sb[:], wo_p[:].rearrange("(hb p) n -> p hb n", p=P))

        # ================= attention =================
        with tc.tile_pool(name="pt_pool", bufs=2) as pt_pool, \
             tc.tile_pool(name="astage", bufs=2) as astage, \
             tc.tile_pool(name="aps", bufs=1, space="PSUM") as aps:
            for s in range(NSLOT):
                Es = E[s]
                for g in range(KVH):
                    qmov = qT[:, g, s, :, :]            # [d, (h t)] moving
                    pT = pt_pool.tile([P, Es, GPQ * P], BF16, name="pT", tag="pT")
                    zp = aps.tile([P, 512], FP32, name="zp", tag="zp")
                    # clear stale PSUM in the rows ones97 will read but the
                    # Z chains do not write (0 * NaN would poison the sum)
                    nc.vector.memset(zp[0:97, :], 0.0)
                    pctx = aps.tile([P, GPQ * P], FP32, name="pctx", tag="pctx")
                    for (c0, blk, use_a) in _batches(Es):
                        if use_a:
                            sc = aps.tile([P, 2048], FP32, name="scA", tag="scA")
                        else:
                            sc = aps.tile([P, 1024], FP32, name="scB", tag="scB")
                        scv = sc[:].rearrange("p (b n) -> p b n", n=512)
                        for i in range(blk):
                            c = c0 + i
                            nc.tensor.matmul(scv[:, i, :],
                                             kT[:, g, c // 4, c % 4, :], qmov,
                                             start=True, stop=True)
                        nc.scalar.activation(
                            pT[:, c0:c0 + blk, :], scv[:, :blk, :], Exp)
                        for i in range(blk):
                            c = c0 + i
                            mi = mask_idx.get((s, c))
                            if mi is not None:
                                pslice = pT[:, c, :].rearrange(
                                    "p (h t) -> p h t", t=P)
                                nc.vector.tensor_tensor(
                                    pslice, pslice,
                                    maskbin[:, mi, None, :]
                                    .to_broadcast((P, GPQ, P)),
                                    mybir.AluOpType.mult)
                        for i in range(blk):
                            c = c0 + i
                            pos = c % 4
                            nc.tensor.matmul(
                                zp[32 * pos:32 * pos + 1, :], ones1[:],
                                pT[:, c, :],
                                start=(c < 4), stop=(c + 4 >= Es),
                                tile_position=(0, 32 * pos))
                        for i in range(blk):
                            c = c0 + i
                            nc.tensor.matmul(
                                pctx[:], v_all[:, c, g * HD:(g + 1) * HD],
                                pT[:, c, :],
                                start=(c == 0), stop=(c == Es - 1))
                    zsb = astage.tile([97, 512], BF16, name="zsb", tag="zsb")
                    nc.vector.tensor_copy(zsb[:], zp[0:97, :])
                    zb = aps.tile([P, 1024], FP32, name="zb", tag="scB")
                    nc.tensor.matmul(zb[:, :512], ones97[:], zsb[:],
                                     start=True, stop=True)
                    rz = astage.tile([P, 512], FP32, name="rz", tag="rz")
                    nc.vector.reciprocal(rz[:], zb[:, :512])
                    nc.vector.tensor_tensor(
                        ctxT[:, s, g * GPQ:(g + 1) * GPQ, :]
                        .rearrange("p h t -> p (h t)"),
                        pctx[:], rz[:], mybir.AluOpType.mult)

        # ================= output projection =================
        with tc.tile_pool(name="ostage", bufs=3) as ostage, \
             tc.tile_pool(name="ops", bufs=1, space="PSUM") as ops:
            for oc in range(HID // 512):
                for s in range(NSLOT):
                    po = ops.tile([P, 512], FP32, name="po", tag=f"po{s % 2}")
                    for hh in range(NH):
                        nc.tensor.matmul(po[:], ctxT[:, s, hh, :],
                                         wo_sb[:, hh, oc * 512:(oc + 1) * 512],
                                         start=(hh == 0), stop=(hh == NH - 1))
                    ot = ostage.tile([P, 512], FP32, name="ot", tag=f"ot{s % 2}")
                    nc.scalar.copy(ot[:], po[:])
                    nc.sync.dma_start(
                        out_p[s * P:(s + 1) * P, oc * 512:(oc + 1) * 512], ot[:])

    nc.compile()
    return nc


def _prep_inputs(hidden_states, attention_mask, cos, sin, Wq, Wk, Wv, Wo, P_list):
    bf16 = ml_dtypes.bfloat16
    hs = np.asarray(hidden_states, dtype=np.float32)
    mask = np.asarray(attention_mask, dtype=np.float32).reshape(T, T)
    cos2 = np.asarray(cos, dtype=np.float32).reshape(T, HD)
    sin2 = np.asarray(sin, dtype=np.float32).reshape(T, HD)
    scale = np.float32(1.0 / np.sqrt(HD))

    def t3(s_):
        # rotate_half add trick: t3 = concat(sin[:, 64:], -sin[:, :64])
        return np.concatenate([s_[:, HD // 2:], -s_[:, :HD // 2]], axis=1)

    wq = np.ascontiguousarray(np.asarray(Wq, dtype=np.float32)).astype(bf16)
    wk = np.ascontiguousarray(np.asarray(Wk, dtype=np.float32)).astype(bf16)
    wv = np.ascontiguousarray(np.asarray(Wv, dtype=np.float32)).astype(bf16)
    wo = np.ascontiguousarray(np.asarray(Wo, dtype=np.float32)).astype(bf16)

    in_maps = []
    for i in range(NC):
        b, pos = i // 4, i % 4
        js = [4 * s + 3 - pos for s in range(NSLOT)]
        take = lambda a: np.ascontiguousarray(
            np.concatenate([a[j * P:(j + 1) * P] for j in js], axis=0))
        xT_rows = take(hs[b])                     # [512, 2048]
        xT = np.ascontiguousarray(xT_rows.T).astype(bf16)
        m_tiles = [
            (mask[js[s] * P:(js[s] + 1) * P,
                  _c_of_j(c) * P:(_c_of_j(c) + 1) * P].T > NEG_THRESH)
            .astype(np.float32)
            for (s, c) in P_list]
        if not m_tiles:
            m_tiles.append(np.ones((P, P), np.float32))
        in_maps.append({
            "xT": xT,
            "wq": wq, "wk": wk, "wv": wv, "wo": wo,
            "cosq": take(cos2 * scale).astype(bf16),
            "sinq3": take(t3(sin2 * scale)).astype(bf16),
            "cosk": take(cos2).astype(bf16),
            "sink3": take(t3(sin2)).astype(bf16),
            "maskbin": np.stack(m_tiles).astype(bf16),
        })
    return in_maps


_cache = {}


def kernel(hidden_states, attention_mask, cos, sin, Wq, Wk, Wv, Wo,
           _trace=False, _trace_kwargs=None):
    from concourse.bass_utils import run_bass_kernel_spmd

    E, P_list = _mask_plan(attention_mask)
    key = (tuple(E), tuple(P_list))
    if key not in _cache:
        _cache[key] = _build_program(E, P_list)
    nc = _cache[key]

    in_maps = _prep_inputs(hidden_states, attention_mask, cos, sin,
                           Wq, Wk, Wv, Wo, P_list)
    kwargs = dict(_trace_kwargs or {})
    if _trace:
        kwargs["trace"] = True
    res = run_bass_kernel_spmd(nc, in_maps, list(range(NC)), **kwargs)

    out = np.empty((B, T, HID), dtype=np.float32)
    for i in range(NC):
        b, pos = i // 4, i % 4
        o = res.results[i]["out"]
        for s in range(NSLOT):
            j = 4 * s + 3 - pos
            out[b, j * P:(j + 1) * P, :] = o[s * P:(s + 1) * P, :]
    kernel._last_result = res
    return out


# revision 15
# speedup vs baseline: 1.0147x; 1.0147x over previous
# Trainium2 Bass kernel for AvaAttention (GQA attention + RoPE + additive mask)
# B=2, T=2048, HID=2048, NH=16, KVH=4, HD=128, fp32 in/out — 8 NeuronCores.
#
# Sharding: sequence-parallel. Core i (batch b=i//4, position p=i%4) owns
# q-blocks j = 4s+3-p of batch b, for slot s in 0..3. Projections are
# row-parallel (weights replicated, host-cast to bf16), K/V exchanged with
# an AllGather over each batch's 4 cores, attention + output projection
# stay local to the core's rows.
#
# v2 design (vs the earlier kernel):
#  - hidden_states arrives pre-transposed from the host (xT), removing the
#    on-device transpose phase entirely.
#  - Attention computes scores directly in [src, (head q)] orientation
#    (stationary = kT block, moving = qT), so the probability tiles come
#    out of exp already transposed for the PV matmul: no per-tile
#    probability transpose, no diag matmuls, no P^T copies on vector.
#  - Softmax denominator Z is accumulated with col-tiled M=1 matmuls
#    (4 concurrent positions at PSUM partitions 0/32/64/96), then one
#    masked-ones matmul sums the partials and broadcasts them across all
#    128 partitions in a single step; a vector reciprocal + one
#    tensor_tensor multiply normalizes ctx while casting to bf16.
#  - The causal mask is applied as a 0/1 multiply on the probability
#    tiles (cheap vector op) instead of -1e9 adds via identity matmuls.
#  - exp runs on the scalar engine in [128, 2048]/[128, 1024] PSUM
#    batches to amortize the per-instruction overhead.
#  - RoPE runs in bf16 on the vector engine (PSUM->bf16 staging copies on
#    the scalar engine) for 2x DVE throughput.

import sys

for _p in ("/opt/trn_rl_repo", "/opt/pypackages"):
    if _p not in sys.path:
        sys.path.insert(0, _p)

import numpy as np
import ml_dtypes

B, T, HID = 2, 2048, 2048
NH, KVH, HD = 16, 4, 128
P = 128
NC = 8
NBLK = T // P          # 16 q-blocks per batch
NSLOT = 4              # blocks per core
GPQ = NH // KVH        # 4 q-heads per kv group
HB = HID // P          # 16 contraction subtiles
NEG_THRESH = -1.0e8


def _c_of_j(j):
    # producer-permuted column-block index (involution)
    return 4 * (j // 4) + 3 - (j % 4)


def _mask_plan(attention_mask):
    """Classify the additive mask per (j, kb) 128x128 tile.

    Returns (E, P_list): E[s] is the uniform extent (in permuted blocks c)
    for slot s; P_list is the ordered list of (s, c) positions where a
    0/1 mask multiply is applied (positions shared by every core; tile
    *data* is per-core).
    """
    m = np.asarray(attention_mask).reshape(T, T)
    nonzero = np.zeros((NBLK, NBLK), dtype=bool)
    live = np.zeros((NBLK, NBLK), dtype=bool)   # not fully masked
    for j in range(NBLK):
        for kb in range(NBLK):
            tile = m[j * P:(j + 1) * P, kb * P:(kb + 1) * P]
            nonzero[j, kb] = bool(np.any(tile != 0.0))
            live[j, kb] = bool(np.any(tile > NEG_THRESH))
    E = []
    for s in range(NSLOT):
        cmax = 1
        for jj in range(4):
            j = 4 * s + jj
            idx = np.nonzero(live[j])[0]
            if len(idx):
                cmax = max(cmax, max(_c_of_j(int(kb)) for kb in idx) + 1)
        E.append(cmax)
    P_list = []
    for s in range(NSLOT):
        for c in range(E[s]):
            kb = _c_of_j(c)
            if any(nonzero[4 * s + jj, kb] for jj in range(4)):
                P_list.append((s, c))
    return E, P_list


def _batches(Es):
    """Split Es blocks into 2-block exp batches alternating tile A / tile B."""
    out = []
    start = 0
    use_a = True
    while start < Es:
        take = min(2, Es - start)
        out.append((start, take, use_a))
        start += take
        use_a = not use_a
    return out


def _build_program(E, P_list):
    import concourse.mybir as mybir
    import concourse.tile as tile
    from concourse import bacc
    from concourse.masks import make_identity
    from contextlib import ExitStack

    FP32 = mybir.dt.float32
    BF16 = mybir.dt.bfloat16
    Exp = mybir.ActivationFunctionType.Exp
    HALF = HD // 2

    nc = bacc.Bacc("TRN2", target_bir_lowering=False, num_devices=NC)

    xT_p = nc.declare_dram_parameter("xT", [HID, NSLOT * P], BF16, isOutput=False)
    wq_p = nc.declare_dram_parameter("wq", [HID, NH * HD], BF16, isOutput=False)
    wk_p = nc.declare_dram_parameter("wk", [HID, KVH * HD], BF16, isOutput=False)
    wv_p = nc.declare_dram_parameter("wv", [HID, KVH * HD], BF16, isOutput=False)
    wo_p = nc.declare_dram_parameter("wo", [HID, HID], BF16, isOutput=False)
    cosq_p = nc.declare_dram_parameter("cosq", [NSLOT * P, HD], BF16, isOutput=False)
    sinq_p = nc.declare_dram_parameter("sinq3", [NSLOT * P, HD], BF16, isOutput=False)
    cosk_p = nc.declare_dram_parameter("cosk", [NSLOT * P, HD], BF16, isOutput=False)
    sink_p = nc.declare_dram_parameter("sink3", [NSLOT * P, HD], BF16, isOutput=False)
    nmask = max(1, len(P_list))
    masks_p = nc.declare_dram_parameter("maskbin", [nmask, P, P], BF16,
                                        isOutput=False)
    out_p = nc.declare_dram_parameter("out", [NSLOT * P, HID], FP32, isOutput=True)

    KVW = KVH * HD  # 512
    ag_k_in = nc.dram_tensor("ag_k_in", [KVW, NSLOT * P], BF16)
    ag_k_out = nc.dram_tensor("ag_k_out", [4, KVW, NSLOT * P], BF16,
                              addr_space="Local")
    ag_v_in = nc.dram_tensor("ag_v_in", [NSLOT * P, KVW], BF16)
    ag_v_out = nc.dram_tensor("ag_v_out", [4, NSLOT * P, KVW], BF16,
                              addr_space="Local")
    groups = [[0, 1, 2, 3], [4, 5, 6, 7]]

    mask_idx = {sk: idx for idx, sk in enumerate(P_list)}

    with tile.TileContext(nc) as tc, ExitStack() as top:
        const = top.enter_context(tc.tile_pool(name="const", bufs=1))
        ident_bf = const.tile([P, P], BF16)
        make_identity(nc, ident_bf[:])
        ones1 = const.tile([P, 1], BF16)
        nc.vector.memset(ones1[:], 1.0)
        ones97 = const.tile([97, P], BF16)
        nc.vector.memset(ones97[:], 0.0)
        for r in (0, 32, 64, 96):
            nc.vector.memset(ones97[r:r + 1, :], 1.0)
        maskbin = const.tile([P, nmask, P], BF16)

        # long-lived attention inputs
        qT_pool = top.enter_context(tc.tile_pool(name="qT_pool", bufs=1))
        qT = qT_pool.tile([P, KVH, NSLOT, GPQ, P], BF16)   # [d, g, s, h, t]

        # ================= projection phase =================
        with tc.tile_pool(name="xw_pool", bufs=1) as xw_pool, \
             tc.tile_pool(name="qw", bufs=2) as qw_pool, \
             tc.tile_pool(name="stage", bufs=2) as stage, \
             tc.tile_pool(name="ropec", bufs=1) as ropec, \
             tc.tile_pool(name="pps", bufs=1, space="PSUM") as pps:
            xT = xw_pool.tile([P, HB, NSLOT * P], BF16)    # [h%128, hb, t]
            wk_sb = xw_pool.tile([P, HB, KVW], BF16)
            wv_sb = xw_pool.tile([P, HB, KVW], BF16)
            xTv = xT_p[:].rearrange("(hb p) t -> p hb t", p=P)
            for hq in range(4):
                nc.sync.dma_start(xT[:, 4 * hq:4 * (hq + 1), :],
                                  xTv[:, 4 * hq:4 * (hq + 1), :])
            nc.sync.dma_start(wk_sb[:], wk_p[:].rearrange("(hb p) n -> p hb n", p=P))
            nc.sync.dma_start(wv_sb[:], wv_p[:].rearrange("(hb p) n -> p hb n", p=P))
            cosq_t = ropec.tile([P, NSLOT, HD], BF16)
            sinq_t = ropec.tile([P, NSLOT, HD], BF16)
            cosk_t = ropec.tile([P, NSLOT, HD], BF16)
            sink_t = ropec.tile([P, NSLOT, HD], BF16)
            for ap, prm in ((cosk_t, cosk_p), (sink_t, sink_p),
                            (cosq_t, cosq_p), (sinq_t, sinq_p)):
                nc.sync.dma_start(ap[:], prm[:].rearrange("(s p) d -> p s d", p=P))
            nc.sync.dma_start(maskbin[:], masks_p[:].rearrange("n p d -> p n d"))
            # prefetch the first two Wq chunks
            wq_tiles = {}
            for hc in range(2):
                w = qw_pool.tile([P, HB, GPQ * HD], BF16, name="wq_sb", tag="wq")
                nc.sync.dma_start(
                    w[:], wq_p[:, hc * GPQ * HD:(hc + 1) * GPQ * HD]
                    .rearrange("(hb p) n -> p hb n", p=P))
                wq_tiles[hc] = w

            def rope(dst, src_sb, cos_t, sin_t, s, nh):
                src3 = src_sb[:].rearrange("p (h d) -> p h d", d=HD)
                cst = stage.tile([P, nh, HD], BF16, name="rope_c", tag="rope_c")
                nc.vector.tensor_tensor(
                    dst[:], src3,
                    cos_t[:, s, None, :].to_broadcast((P, nh, HD)),
                    mybir.AluOpType.mult)
                nc.vector.tensor_tensor(
                    cst[:], src3,
                    sin_t[:, s, None, :].to_broadcast((P, nh, HD)),
                    mybir.AluOpType.mult)
                nc.vector.tensor_tensor(dst[:, :, HALF:], dst[:, :, HALF:],
                                        cst[:, :, :HALF], mybir.AluOpType.add)
                nc.vector.tensor_tensor(dst[:, :, :HALF], dst[:, :, :HALF],
                                        cst[:, :, HALF:], mybir.AluOpType.add)

            # ---- K projection + rope + transpose + AllGather ----
            pk = [pps.tile([P, KVW], FP32, name=f"pk{s}", tag=f"pkv{s}")
                  for s in range(NSLOT)]
            for hb in range(HB):
                for s in range(NSLOT):
                    nc.tensor.matmul(pk[s][:], xT[:, hb, s * P:(s + 1) * P],
                                     wk_sb[:, hb, :],
                                     start=(hb == 0), stop=(hb == HB - 1))
            k_rope = []
            for s in range(NSLOT):
                kst = stage.tile([P, KVW], BF16, name=f"kst{s}", tag=f"kst{s % 2}")
                nc.scalar.copy(kst[:], pk[s][:])
                kr = xw_pool.tile([P, KVH, HD], BF16, name=f"k_rope{s}")
                rope(kr, kst, cosk_t, sink_t, s, KVH)
                k_rope.append(kr)
            contrib_k = xw_pool.tile([P, KVH, NSLOT * P], BF16, name="contrib_k")
            for g in range(KVH):
                pkt = pps.tile([P, NSLOT * P], BF16, name="pkt", tag=f"pkt{g % 2}")
                for s in range(NSLOT):
                    nc.tensor.transpose(pkt[:, s * P:(s + 1) * P],
                                        k_rope[s][:, g, :], ident_bf[:])
                nc.vector.tensor_copy(contrib_k[:, g, :], pkt[:])
            nc.sync.dma_start(
                ag_k_in[:].rearrange("(g d) t -> d g t", d=P), contrib_k[:])
            nc.gpsimd.collective_compute(
                "AllGather", mybir.AluOpType.bypass, replica_groups=groups,
                ins=[ag_k_in[:]], outs=[ag_k_out[:]])

            # ---- V projection + AllGather ----
            pv = [pps.tile([P, KVW], FP32, name=f"pv{s}", tag=f"pkv{s}")
                  for s in range(NSLOT)]
            for hb in range(HB):
                for s in range(NSLOT):
                    nc.tensor.matmul(pv[s][:], xT[:, hb, s * P:(s + 1) * P],
                                     wv_sb[:, hb, :],
                                     start=(hb == 0), stop=(hb == HB - 1))
            for s in range(NSLOT):
                vst = stage.tile([P, KVW], BF16, name=f"vst{s}", tag="vst")
                nc.scalar.copy(vst[:], pv[s][:])
                nc.sync.dma_start(ag_v_in[s * P:(s + 1) * P, :], vst[:])
            nc.gpsimd.collective_compute(
                "AllGather", mybir.AluOpType.bypass, replica_groups=groups,
                ins=[ag_v_in[:]], outs=[ag_v_out[:]])

            # ---- Q projection + rope + transpose to qT ----
            for hc in range(NH // GPQ):
                if hc in wq_tiles:
                    wq_sb = wq_tiles[hc]
                else:
                    wq_sb = qw_pool.tile([P, HB, GPQ * HD], BF16,
                                         name="wq_sb", tag="wq")
                    nc.sync.dma_start(
                        wq_sb[:], wq_p[:, hc * GPQ * HD:(hc + 1) * GPQ * HD]
                        .rearrange("(hb p) n -> p hb n", p=P))
                pq = [pps.tile([P, GPQ * HD], FP32, name=f"pq{s}", tag=f"pkv{s}")
                      for s in range(NSLOT)]
                for hb in range(HB):
                    for s in range(NSLOT):
                        nc.tensor.matmul(pq[s][:], xT[:, hb, s * P:(s + 1) * P],
                                         wq_sb[:, hb, :],
                                         start=(hb == 0), stop=(hb == HB - 1))
                for s in range(NSLOT):
                    qst = stage.tile([P, GPQ * HD], BF16, name="qst",
                                     tag=f"qst{s % 2}")
                    nc.scalar.copy(qst[:], pq[s][:])
                    qr = stage.tile([P, GPQ, HD], BF16, name="q_rope",
                                    tag=f"q_rope{s % 2}")
                    rope(qr, qst, cosq_t, sinq_t, s, GPQ)
                    pqt = pps.tile([P, GPQ * P], BF16, name="pqt",
                                   tag=f"pqt{s % 2}")
                    for h in range(GPQ):
                        nc.tensor.transpose(pqt[:, h * P:(h + 1) * P],
                                            qr[:, h, :], ident_bf[:])
                    nc.vector.tensor_copy(
                        qT[:, hc, s, :, :],
                        pqt[:].rearrange("p (h t) -> p h t", t=P))

        # ---- gather K/V from the AllGather outputs; preload wo ----
        # (pool created after the projection pools release their SBUF)
        kv_pool = top.enter_context(tc.tile_pool(name="kv_pool", bufs=1))
        # c = 4*s_src + pos is addressed as [s_src, pos]
        kT = kv_pool.tile([P, KVH, NSLOT, 4, P], BF16)     # [d, g, s_src, pos, t]
        v_all = kv_pool.tile([P, NBLK, KVW], BF16)         # [t%128, c, (g d)]
        ctxT = kv_pool.tile([P, NSLOT, NH, P], BF16)       # [d, s, hh, t]
        wo_sb = kv_pool.tile([P, HB, HID], BF16)
        for g in range(KVH):
            for pos in range(4):
                nc.sync.dma_start(
                    kT[:, g, :, pos, :],
                    ag_k_out[pos, g * P:(g + 1) * P, :]
                    .rearrange("d (s t) -> d s t", t=P))
        for s_src in range(NSLOT):
            for pos in range(4):
                nc.sync.dma_start(v_all[:, 4 * s_src + pos, :],
                                  ag_v_out[pos, s_src * P:(s_src + 1) * P, :])
        nc.sync.dma_start(wo_sb[:], wo_p[:].rearrange("(hb p) n -> p hb n", p=P))

        # ============ attention + interleaved output projection ============
        # The finalize (Z sum/broadcast, reciprocal, ctx normalize) of group
        # (s, g) is deferred until after the next group's first score batch
        # so the in-order tensor queue never waits on the vector engine.
        # Output-projection chains for slot s-1 are injected between a
        # group's exp and its Z/PV matmuls: they fill the tensor queue while
        # the scalar engine runs exp.
        with tc.tile_pool(name="pt_pool", bufs=2) as pt_pool, \
             tc.tile_pool(name="astage", bufs=2) as astage, \
             tc.tile_pool(name="ostage", bufs=2) as ostage, \
             tc.tile_pool(name="aps", bufs=1, space="PSUM") as aps:

            def make_finalize(s, g, zp, pctx):
                def fin():
                    zsb = astage.tile([97, 512], BF16, name="zsb", tag="zsb")
                    nc.vector.tensor_copy(zsb[:], zp[0:97, :])
                    zb = aps.tile([P, 1024], FP32, name="zb", tag="scB")
                    nc.tensor.matmul(zb[:, :512], ones97[:], zsb[:],
                                     start=True, stop=True)
                    rz = astage.tile([P, 512], FP32, name="rz", tag="rz")
                    nc.vector.reciprocal(rz[:], zb[:, :512])
                    nc.vector.tensor_tensor(
                        ctxT[:, s, g * GPQ:(g + 1) * GPQ, :]
                        .rearrange("p h t -> p (h t)"),
                        pctx[:], rz[:], mybir.AluOpType.mult)
                return fin

            def emit_po(s, oc):
                po = aps.tile([P, 512], FP32, name="po", tag="po")
                for hh in range(NH):
                    nc.tensor.matmul(po[:], ctxT[:, s, hh, :],
                                     wo_sb[:, hh, oc * 512:(oc + 1) * 512],
                                     start=(hh == 0), stop=(hh == NH - 1))
                ot = ostage.tile([P, 512], FP32, name="ot", tag="ot")
                nc.vector.tensor_copy(ot[:], po[:])
                nc.sync.dma_start(
                    out_p[s * P:(s + 1) * P, oc * 512:(oc + 1) * 512], ot[:])

            pending_fin = None
            po_queue = []
            for s in range(NSLOT):
                Es = E[s]
                for g in range(KVH):
                    qmov = qT[:, g, s, :, :]            # [d, (h t)] moving
                    pT = pt_pool.tile([P, Es, GPQ * P], BF16, name="pT", tag="pT")
                    zp = aps.tile([P, 512], FP32, name="zp", tag="zp")
                    # clear stale PSUM in the rows ones97 will read but the
                    # Z chains do not write (0 * NaN would poison the sum)
                    nc.vector.memset(zp[0:97, :], 0.0)
                    pctx = aps.tile([P, GPQ * P], FP32, name="pctx", tag="pctx")
                    for (c0, blk, use_a) in _batches(Es):
                        sc = aps.tile([P, 1024], FP32, name="sc",
                                      tag="scA" if use_a else "scB")
                        scv = sc[:].rearrange("p (b n) -> p b n", n=512)
                        for i in range(blk):
                            c = c0 + i
                            nc.tensor.matmul(scv[:, i, :],
                                             kT[:, g, c // 4, c % 4, :], qmov,
                                             start=True, stop=True)
                        nc.scalar.activation(
                            pT[:, c0:c0 + blk, :], scv[:, :blk, :], Exp)
                        if c0 == 0 and pending_fin is not None:
                            pending_fin()
                            pending_fin = None
                        if c0 == 2 and po_queue:
                            emit_po(*po_queue.pop(0))
                        for i in range(blk):
                            c = c0 + i
                            mi = mask_idx.get((s, c))
                            if mi is not None:
                                pslice = pT[:, c, :].rearrange(
                                    "p (h t) -> p h t", t=P)
                                nc.vector.tensor_tensor(
                                    pslice, pslice,
                                    maskbin[:, mi, None, :]
                                    .to_broadcast((P, GPQ, P)),
                                    mybir.AluOpType.mult)
                        for i in range(blk):
                            c = c0 + i
                            pos = c % 4
                            nc.tensor.matmul(
                                zp[32 * pos:32 * pos + 1, :], ones1[:],
                                pT[:, c, :],
                                start=(c < 4), stop=(c + 4 >= Es),
                                tile_position=(0, 32 * pos))
                        for i in range(blk):
                            c = c0 + i
                            nc.tensor.matmul(
                                pctx[:], v_all[:, c, g * HD:(g + 1) * HD],
                                pT[:, c, :],
                                start=(c == 0), stop=(c == Es - 1))
                    pending_fin = make_finalize(s, g, zp, pctx)
                po_queue.extend((s, oc) for oc in range(HID // 512))
            if pending_fin is not None:
                pending_fin()
            for so in po_queue:
                emit_po(*so)

    nc.compile()
    return nc


def _prep_inputs(hidden_states, attention_mask, cos, sin, Wq, Wk, Wv, Wo, P_list):
    bf16 = ml_dtypes.bfloat16
    hs = np.asarray(hidden_states, dtype=np.float32)
    mask = np.asarray(attention_mask, dtype=np.float32).reshape(T, T)
    cos2 = np.asarray(cos, dtype=np.float32).reshape(T, HD)
    sin2 = np.asarray(sin, dtype=np.float32).reshape(T, HD)
    scale = np.float32(1.0 / np.sqrt(HD))

    def t3(s_):
        # rotate_half add trick: t3 = concat(sin[:, 64:], -sin[:, :64])
        return np.concatenate([s_[:, HD // 2:], -s_[:, :HD // 2]], axis=1)

    wq = np.ascontiguousarray(np.asarray(Wq, dtype=np.float32)).astype(bf16)
    wk = np.ascontiguousarray(np.asarray(Wk, dtype=np.float32)).astype(bf16)
    wv = np.ascontiguousarray(np.asarray(Wv, dtype=np.float32)).astype(bf16)
    wo = np.ascontiguousarray(np.asarray(Wo, dtype=np.float32)).astype(bf16)

    in_maps = []
    for i in range(NC):
        b, pos = i // 4, i % 4
        js = [4 * s + 3 - pos for s in range(NSLOT)]
        take = lambda a: np.ascontiguousarray(
            np.concatenate([a[j * P:(j + 1) * P] for j in js], axis=0))
        xT_rows = take(hs[b])                     # [512, 2048]
        xT = np.ascontiguousarray(xT_rows.T).astype(bf16)
        m_tiles = [
            (mask[js[s] * P:(js[s] + 1) * P,
                  _c_of_j(c) * P:(_c_of_j(c) + 1) * P].T > NEG_THRESH)
            .astype(np.float32)
            for (s, c) in P_list]
        if not m_tiles:
            m_tiles.append(np.ones((P, P), np.float32))
        in_maps.append({
            "xT": xT,
            "wq": wq, "wk": wk, "wv": wv, "wo": wo,
            "cosq": take(cos2 * scale).astype(bf16),
            "sinq3": take(t3(sin2 * scale)).astype(bf16),
            "cosk": take(cos2).astype(bf16),
            "sink3": take(t3(sin2)).astype(bf16),
            "maskbin": np.stack(m_tiles).astype(bf16),
        })
    return in_maps


_cache = {}


def kernel(hidden_states, attention_mask, cos, sin, Wq, Wk, Wv, Wo,
           _trace=False, _trace_kwargs=None):
    from concourse.bass_utils import run_bass_kernel_spmd

    E, P_list = _mask_plan(attention_mask)
    key = (tuple(E), tuple(P_list))
    if key not in _cache:
        _cache[key] = _build_program(E, P_list)
    nc = _cache[key]

    in_maps = _prep_inputs(hidden_states, attention_mask, cos, sin,
                           Wq, Wk, Wv, Wo, P_list)
    kwargs = dict(_trace_kwargs or {})
    if _trace:
        kwargs["trace"] = True
    res = run_bass_kernel_spmd(nc, in_maps, list(range(NC)), **kwargs)

    out = np.empty((B, T, HID), dtype=np.float32)
    for i in range(NC):
        b, pos = i // 4, i % 4
        o = res.results[i]["out"]
        for s in range(NSLOT):
            j = 4 * s + 3 - pos
            out[b, j * P:(j + 1) * P, :] = o[s * P:(s + 1) * P, :]
    kernel._last_result = res
    return out


# revision 20
# speedup vs baseline: 1.0992x; 1.0832x over previous
# Trainium2 Bass kernel for AvaAttention (GQA attention + RoPE + additive mask)
# B=2, T=2048, HID=2048, NH=16, KVH=4, HD=128, fp32 in/out — 8 NeuronCores.
#
# Sharding: sequence-parallel. Core i (batch b=i//4, position p=i%4) owns
# q-blocks j = 4s+3-p of batch b, for slot s in 0..3. Projections are
# row-parallel (weights replicated, host-cast to bf16), K/V exchanged with
# an AllGather over each batch's 4 cores, attention + output projection
# stay local to the core's rows.
#
# v2 design (vs the earlier kernel):
#  - hidden_states arrives pre-transposed from the host (xT), removing the
#    on-device transpose phase entirely.
#  - Attention computes scores directly in [src, (head q)] orientation
#    (stationary = kT block, moving = qT), so the probability tiles come
#    out of exp already transposed for the PV matmul: no per-tile
#    probability transpose, no diag matmuls, no P^T copies on vector.
#  - Softmax denominator Z is accumulated with col-tiled M=1 matmuls
#    (4 concurrent positions at PSUM partitions 0/32/64/96), then one
#    masked-ones matmul sums the partials and broadcasts them across all
#    128 partitions in a single step; a vector reciprocal + one
#    tensor_tensor multiply normalizes ctx while casting to bf16.
#  - The causal mask is applied as a 0/1 multiply on the probability
#    tiles (cheap vector op) instead of -1e9 adds via identity matmuls.
#  - exp runs on the scalar engine in [128, 2048]/[128, 1024] PSUM
#    batches to amortize the per-instruction overhead.
#  - RoPE runs in bf16 on the vector engine (PSUM->bf16 staging copies on
#    the scalar engine) for 2x DVE throughput.

import sys

for _p in ("/opt/trn_rl_repo", "/opt/pypackages"):
    if _p not in sys.path:
        sys.path.insert(0, _p)

import numpy as np
import ml_dtypes

B, T, HID = 2, 2048, 2048
NH, KVH, HD = 16, 4, 128
P = 128
NC = 8
NBLK = T // P          # 16 q-blocks per batch
NSLOT = 4              # blocks per core
GPQ = NH // KVH        # 4 q-heads per kv group
HB = HID // P          # 16 contraction subtiles
NEG_THRESH = -1.0e8


def _c_of_j(j):
    # producer-permuted column-block index (involution)
    return 4 * (j // 4) + 3 - (j % 4)


def _mask_plan(attention_mask):
    """Classify the additive mask per (j, kb) 128x128 tile.

    Returns (E, P_list): E[s] is the uniform extent (in permuted blocks c)
    for slot s; P_list is the ordered list of (s, c) positions where a
    0/1 mask multiply is applied (positions shared by every core; tile
    *data* is per-core).
    """
    m = np.asarray(attention_mask).reshape(T, T)
    nonzero = np.zeros((NBLK, NBLK), dtype=bool)
    live = np.zeros((NBLK, NBLK), dtype=bool)   # not fully masked
    for j in range(NBLK):
        for kb in range(NBLK):
            tile = m[j * P:(j + 1) * P, kb * P:(kb + 1) * P]
            nonzero[j, kb] = bool(np.any(tile != 0.0))
            live[j, kb] = bool(np.any(tile > NEG_THRESH))
    E = []
    for s in range(NSLOT):
        cmax = 1
        for jj in range(4):
            j = 4 * s + jj
            idx = np.nonzero(live[j])[0]
            if len(idx):
                cmax = max(cmax, max(_c_of_j(int(kb)) for kb in idx) + 1)
        E.append(cmax)
    P_list = []
    for s in range(NSLOT):
        for c in range(E[s]):
            kb = _c_of_j(c)
            if any(nonzero[4 * s + jj, kb] for jj in range(4)):
                P_list.append((s, c))
    return E, P_list


def _batches(Es):
    """Split Es blocks into 2-block exp batches alternating tile A / tile B."""
    out = []
    start = 0
    use_a = True
    while start < Es:
        take = min(2, Es - start)
        out.append((start, take, use_a))
        start += take
        use_a = not use_a
    return out


def _build_program(E, P_list):
    import concourse.mybir as mybir
    import concourse.tile as tile
    from concourse import bacc
    from concourse.masks import make_identity
    from contextlib import ExitStack

    FP32 = mybir.dt.float32
    BF16 = mybir.dt.bfloat16
    Exp = mybir.ActivationFunctionType.Exp
    HALF = HD // 2

    nc = bacc.Bacc("TRN2", target_bir_lowering=False, num_devices=NC)

    xT_p = nc.declare_dram_parameter("xT", [HID, NSLOT * P], BF16, isOutput=False)
    wq_p = nc.declare_dram_parameter("wq", [HID, NH * HD], BF16, isOutput=False)
    wk_p = nc.declare_dram_parameter("wk", [HID, KVH * HD], BF16, isOutput=False)
    wv_p = nc.declare_dram_parameter("wv", [HID, KVH * HD], BF16, isOutput=False)
    wo_p = nc.declare_dram_parameter("wo", [HID, HID], BF16, isOutput=False)
    cosq_p = nc.declare_dram_parameter("cosq", [NSLOT * P, HD], BF16, isOutput=False)
    sinq_p = nc.declare_dram_parameter("sinq3", [NSLOT * P, HD], BF16, isOutput=False)
    cosk_p = nc.declare_dram_parameter("cosk", [NSLOT * P, HD], BF16, isOutput=False)
    sink_p = nc.declare_dram_parameter("sink3", [NSLOT * P, HD], BF16, isOutput=False)
    nmask = max(1, len(P_list))
    masks_p = nc.declare_dram_parameter("maskbin", [nmask, P, P], BF16,
                                        isOutput=False)
    out_p = nc.declare_dram_parameter("out", [NSLOT * P, HID], FP32, isOutput=True)

    KVW = KVH * HD  # 512
    ag_k_in = nc.dram_tensor("ag_k_in", [KVW, NSLOT * P], BF16)
    ag_k_out = nc.dram_tensor("ag_k_out", [4, KVW, NSLOT * P], BF16,
                              addr_space="Local")
    ag_v_in = nc.dram_tensor("ag_v_in", [NSLOT * P, KVW], BF16)
    ag_v_out = nc.dram_tensor("ag_v_out", [4, NSLOT * P, KVW], BF16,
                              addr_space="Local")
    groups = [[0, 1, 2, 3], [4, 5, 6, 7]]

    mask_idx = {sk: idx for idx, sk in enumerate(P_list)}

    with tile.TileContext(nc) as tc, ExitStack() as top:
        const = top.enter_context(tc.tile_pool(name="const", bufs=1))
        ident_bf = const.tile([P, P], BF16)
        make_identity(nc, ident_bf[:])
        ones1 = const.tile([P, 1], BF16)
        nc.vector.memset(ones1[:], 1.0)
        ones97 = const.tile([97, P], BF16)
        nc.vector.memset(ones97[:], 0.0)
        for r in (0, 32, 64, 96):
            nc.vector.memset(ones97[r:r + 1, :], 1.0)
        maskbin = const.tile([P, nmask, P], BF16)

        # long-lived attention inputs
        qT_pool = top.enter_context(tc.tile_pool(name="qT_pool", bufs=1))
        qT = qT_pool.tile([P, KVH, NSLOT, GPQ, P], BF16)   # [d, g, s, h, t]

        # ================= projection phase =================
        with tc.tile_pool(name="xw_pool", bufs=1) as xw_pool, \
             tc.tile_pool(name="qw", bufs=2) as qw_pool, \
             tc.tile_pool(name="stage", bufs=2) as stage, \
             tc.tile_pool(name="ropec", bufs=1) as ropec, \
             tc.tile_pool(name="pps", bufs=1, space="PSUM") as pps:
            xT = xw_pool.tile([P, HB, NSLOT * P], BF16)    # [h%128, hb, t]
            wk_sb = xw_pool.tile([P, HB, KVW], BF16)
            wv_sb = xw_pool.tile([P, HB, KVW], BF16)
            # interleave xT/wk chunk DMAs so the first K-proj matmuls can
            # start after ~128KB instead of after every input tensor
            xTv = xT_p[:].rearrange("(hb p) t -> p hb t", p=P)
            wkv = wk_p[:].rearrange("(hb p) n -> p hb n", p=P)
            for hq in range(4):
                sl = slice(4 * hq, 4 * (hq + 1))
                nc.sync.dma_start(xT[:, sl, :], xTv[:, sl, :])
                nc.sync.dma_start(wk_sb[:, sl, :], wkv[:, sl, :])
            nc.sync.dma_start(wv_sb[:], wv_p[:].rearrange("(hb p) n -> p hb n", p=P))
            cosq_t = ropec.tile([P, NSLOT, HD], BF16)
            sinq_t = ropec.tile([P, NSLOT, HD], BF16)
            cosk_t = ropec.tile([P, NSLOT, HD], BF16)
            sink_t = ropec.tile([P, NSLOT, HD], BF16)
            for ap, prm in ((cosk_t, cosk_p), (sink_t, sink_p),
                            (cosq_t, cosq_p), (sinq_t, sinq_p)):
                nc.sync.dma_start(ap[:], prm[:].rearrange("(s p) d -> p s d", p=P))
            nc.sync.dma_start(maskbin[:], masks_p[:].rearrange("n p d -> p n d"))
            # prefetch the first two Wq chunks
            wq_tiles = {}
            for hc in range(2):
                w = qw_pool.tile([P, HB, GPQ * HD], BF16, name="wq_sb", tag="wq")
                nc.sync.dma_start(
                    w[:], wq_p[:, hc * GPQ * HD:(hc + 1) * GPQ * HD]
                    .rearrange("(hb p) n -> p hb n", p=P))
                wq_tiles[hc] = w

            def rope(dst, src_sb, cos_t, sin_t, s, nh):
                src3 = src_sb[:].rearrange("p (h d) -> p h d", d=HD)
                cst = stage.tile([P, nh, HD], BF16, name="rope_c", tag="rope_c")
                nc.vector.tensor_tensor(
                    dst[:], src3,
                    cos_t[:, s, None, :].to_broadcast((P, nh, HD)),
                    mybir.AluOpType.mult)
                nc.vector.tensor_tensor(
                    cst[:], src3,
                    sin_t[:, s, None, :].to_broadcast((P, nh, HD)),
                    mybir.AluOpType.mult)
                nc.vector.tensor_tensor(dst[:, :, HALF:], dst[:, :, HALF:],
                                        cst[:, :, :HALF], mybir.AluOpType.add)
                nc.vector.tensor_tensor(dst[:, :, :HALF], dst[:, :, :HALF],
                                        cst[:, :, HALF:], mybir.AluOpType.add)

            # ---- K projection + rope + transpose + AllGather ----
            pk = [pps.tile([P, KVW], FP32, name=f"pk{s}", tag=f"pkv{s}")
                  for s in range(NSLOT)]
            for hb in range(HB):
                for s in range(NSLOT):
                    nc.tensor.matmul(pk[s][:], xT[:, hb, s * P:(s + 1) * P],
                                     wk_sb[:, hb, :],
                                     start=(hb == 0), stop=(hb == HB - 1))
            k_rope = []
            for s in range(NSLOT):
                kst = stage.tile([P, KVW], BF16, name=f"kst{s}", tag=f"kst{s % 2}")
                nc.scalar.copy(kst[:], pk[s][:])
                kr = xw_pool.tile([P, KVH, HD], BF16, name=f"k_rope{s}")
                rope(kr, kst, cosk_t, sink_t, s, KVH)
                k_rope.append(kr)
            contrib_k = xw_pool.tile([P, KVH, NSLOT * P], BF16, name="contrib_k")
            for g in range(KVH):
                pkt = pps.tile([P, NSLOT * P], BF16, name="pkt", tag=f"pkt{g % 2}")
                for s in range(NSLOT):
                    nc.tensor.transpose(pkt[:, s * P:(s + 1) * P],
                                        k_rope[s][:, g, :], ident_bf[:])
                nc.vector.tensor_copy(contrib_k[:, g, :], pkt[:])
            nc.sync.dma_start(
                ag_k_in[:].rearrange("(g d) t -> d g t", d=P), contrib_k[:])
            nc.gpsimd.collective_compute(
                "AllGather", mybir.AluOpType.bypass, replica_groups=groups,
                ins=[ag_k_in[:]], outs=[ag_k_out[:]])

            # ---- V projection + AllGather ----
            pv = [pps.tile([P, KVW], FP32, name=f"pv{s}", tag=f"pkv{s}")
                  for s in range(NSLOT)]
            for hb in range(HB):
                for s in range(NSLOT):
                    nc.tensor.matmul(pv[s][:], xT[:, hb, s * P:(s + 1) * P],
                                     wv_sb[:, hb, :],
                                     start=(hb == 0), stop=(hb == HB - 1))
            for s in range(NSLOT):
                vst = stage.tile([P, KVW], BF16, name=f"vst{s}", tag="vst")
                nc.scalar.copy(vst[:], pv[s][:])
                nc.sync.dma_start(ag_v_in[s * P:(s + 1) * P, :], vst[:])
            nc.gpsimd.collective_compute(
                "AllGather", mybir.AluOpType.bypass, replica_groups=groups,
                ins=[ag_v_in[:]], outs=[ag_v_out[:]])

            # ---- Q projection + rope + transpose to qT ----
            for hc in range(NH // GPQ):
                if hc in wq_tiles:
                    wq_sb = wq_tiles[hc]
                else:
                    wq_sb = qw_pool.tile([P, HB, GPQ * HD], BF16,
                                         name="wq_sb", tag="wq")
                    nc.sync.dma_start(
                        wq_sb[:], wq_p[:, hc * GPQ * HD:(hc + 1) * GPQ * HD]
                        .rearrange("(hb p) n -> p hb n", p=P))
                pq = [pps.tile([P, GPQ * HD], FP32, name=f"pq{s}", tag=f"pkv{s}")
                      for s in range(NSLOT)]
                for hb in range(HB):
                    for s in range(NSLOT):
                        nc.tensor.matmul(pq[s][:], xT[:, hb, s * P:(s + 1) * P],
                                         wq_sb[:, hb, :],
                                         start=(hb == 0), stop=(hb == HB - 1))
                for s in range(NSLOT):
                    qst = stage.tile([P, GPQ * HD], BF16, name="qst",
                                     tag=f"qst{s % 2}")
                    nc.scalar.copy(qst[:], pq[s][:])
                    qr = stage.tile([P, GPQ, HD], BF16, name="q_rope",
                                    tag=f"q_rope{s % 2}")
                    rope(qr, qst, cosq_t, sinq_t, s, GPQ)
                    pqt = pps.tile([P, GPQ * P], BF16, name="pqt",
                                   tag=f"pqt{s % 2}")
                    for h in range(GPQ):
                        nc.tensor.transpose(pqt[:, h * P:(h + 1) * P],
                                            qr[:, h, :], ident_bf[:])
                    nc.vector.tensor_copy(
                        qT[:, hc, s, :, :],
                        pqt[:].rearrange("p (h t) -> p h t", t=P))

        # ---- gather K/V from the AllGather outputs; preload wo ----
        # (pool created after the projection pools release their SBUF)
        kv_pool = top.enter_context(tc.tile_pool(name="kv_pool", bufs=1))
        # c = 4*s_src + pos is addressed as [s_src, pos]
        kT = kv_pool.tile([P, KVH, NSLOT, 4, P], BF16)     # [d, g, s_src, pos, t]
        v_all = kv_pool.tile([P, NBLK, KVW], BF16)         # [t%128, c, (g d)]
        ctxT = kv_pool.tile([P, NSLOT, NH, P], BF16)       # [d, s, hh, t]
        wo_sb = kv_pool.tile([P, HB, HID], BF16)
        for g in range(KVH):
            for pos in range(4):
                nc.sync.dma_start(
                    kT[:, g, :, pos, :],
                    ag_k_out[pos, g * P:(g + 1) * P, :]
                    .rearrange("d (s t) -> d s t", t=P))
        for s_src in range(NSLOT):
            for pos in range(4):
                nc.sync.dma_start(v_all[:, 4 * s_src + pos, :],
                                  ag_v_out[pos, s_src * P:(s_src + 1) * P, :])
        nc.sync.dma_start(wo_sb[:], wo_p[:].rearrange("(hb p) n -> p hb n", p=P))

        # ============ attention + interleaved output projection ============
        # Flat software pipeline over all (slot, group, batch) score batches:
        # the Z/PV consumption of batch n-2 is emitted together with the
        # scores of batch n, so the in-order tensor queue never waits on the
        # scalar engine's exp.  Output-projection chains are split into
        # 4-matmul pieces injected one per pipeline step; the softmax
        # finalize (Z sum/broadcast, reciprocal, normalize) of each group is
        # staged across two steps.  The goal is a gapless tensor stream:
        # any multi-us PE idle drops the PE array to its half-pump state.
        with tc.tile_pool(name="pt_pool", bufs=2) as pt_pool, \
             tc.tile_pool(name="astage", bufs=2) as astage, \
             tc.tile_pool(name="ostage", bufs=2) as ostage, \
             tc.tile_pool(name="aps", bufs=1, space="PSUM") as aps, \
             tc.tile_pool(name="cps", bufs=2, space="PSUM") as cps:

            DELAY = 2
            batches = []      # (s, g, c0, blk, tag, is_first, is_last)
            for s in range(NSLOT):
                for g in range(KVH):
                    bl = _batches(E[s])
                    for bi, (c0, blk, use_a) in enumerate(bl):
                        batches.append((s, g, c0, blk,
                                        "scA" if use_a else "scB",
                                        bi == 0, bi == len(bl) - 1))

            sc_tiles = {}     # stream index -> score psum tile
            group_state = {}  # (s, g) -> dict(pT=, zp=, pctx=)
            po_pieces = []    # queue of (s, oc, piece_idx)
            po_live = {}      # (s, oc) -> psum tile
            fin_b_pending = []

            def emit_scores(n):
                s, g, c0, blk, tag, first, last = batches[n]
                if first:
                    st = group_state[(s, g)] = {}
                    st["pT"] = pt_pool.tile([P, E[s], GPQ * P], BF16,
                                            name="pT", tag="pT")
                sc = aps.tile([P, 1024], FP32, name="sc", tag=tag)
                sc_tiles[n] = sc
                st = group_state[(s, g)]
                scv = sc[:].rearrange("p (b n) -> p b n", n=512)
                qmov = qT[:, g, s, :, :]
                for i in range(blk):
                    c = c0 + i
                    nc.tensor.matmul(scv[:, i, :],
                                     kT[:, g, c // 4, c % 4, :], qmov,
                                     start=True, stop=True)
                pT = st["pT"]
                nc.scalar.activation(pT[:, c0:c0 + blk, :], scv[:, :blk, :], Exp)
                for i in range(blk):
                    c = c0 + i
                    mi = mask_idx.get((s, c))
                    if mi is not None:
                        pslice = pT[:, c, :].rearrange("p (h t) -> p h t", t=P)
                        nc.vector.tensor_tensor(
                            pslice, pslice,
                            maskbin[:, mi, None, :].to_broadcast((P, GPQ, P)),
                            mybir.AluOpType.mult)

            def emit_consume(n):
                s, g, c0, blk, tag, first, last = batches[n]
                Es = E[s]
                st = group_state[(s, g)]
                if first:
                    zp = st["zp"] = aps.tile([P, 512], FP32, name="zp", tag="zp")
                    if (s, g) == (0, 0):
                        # clear stale PSUM once in the rows ones97 reads but
                        # the Z chains never write (0 * NaN would poison the
                        # sum); the zp ring owns this bank, and the unwritten
                        # rows stay zero for the rest of the kernel
                        nc.vector.memset(zp[0:97, :], 0.0)
                    st["pctx"] = cps.tile([P, GPQ * P], FP32,
                                          name="pctx", tag="pctx")
                pT, zp, pctx = st["pT"], st["zp"], st["pctx"]
                for i in range(blk):
                    c = c0 + i
                    pos = c % 4
                    nc.tensor.matmul(
                        zp[32 * pos:32 * pos + 1, :], ones1[:], pT[:, c, :],
                        start=(c < 4), stop=(c + 4 >= Es),
                        tile_position=(0, 32 * pos))
                for i in range(blk):
                    c = c0 + i
                    nc.tensor.matmul(
                        pctx[:], v_all[:, c, g * HD:(g + 1) * HD], pT[:, c, :],
                        start=(c == 0), stop=(c == Es - 1))
                if last:
                    zsb = astage.tile([97, 512], BF16, name="zsb", tag="zsb")
                    nc.vector.tensor_copy(zsb[:], zp[0:97, :])
                    fin_b_pending.append((s, g, zsb, pctx))

            def emit_fin_b():
                s, g, zsb, pctx = fin_b_pending.pop(0)
                zb = aps.tile([P, 1024], FP32, name="zb", tag="scB")
                nc.tensor.matmul(zb[:, :512], ones97[:], zsb[:],
                                 start=True, stop=True)
                rz = astage.tile([P, 512], FP32, name="rz", tag="rz")
                nc.vector.reciprocal(rz[:], zb[:, :512])
                nc.vector.tensor_tensor(
                    ctxT[:, s, g * GPQ:(g + 1) * GPQ, :]
                    .rearrange("p h t -> p (h t)"),
                    pctx[:], rz[:], mybir.AluOpType.mult)
                if g == KVH - 1:
                    po_pieces.extend((s, oc, pc)
                                     for oc in range(HID // 512)
                                     for pc in range(4))

            def emit_po_piece():
                s, oc, pc = po_pieces.pop(0)
                if pc == 0:
                    po_live[(s, oc)] = aps.tile([P, 512], FP32,
                                                name="po", tag="po")
                po = po_live[(s, oc)]
                for hh in range(4 * pc, 4 * pc + 4):
                    nc.tensor.matmul(po[:], ctxT[:, s, hh, :],
                                     wo_sb[:, hh, oc * 512:(oc + 1) * 512],
                                     start=(hh == 0), stop=(hh == NH - 1),
                                     skip_group_check=True)
                if pc == 3:
                    ot = ostage.tile([P, 512], FP32, name="ot", tag="ot")
                    nc.vector.tensor_copy(ot[:], po[:])
                    nc.sync.dma_start(
                        out_p[s * P:(s + 1) * P, oc * 512:(oc + 1) * 512],
                        ot[:])
                    del po_live[(s, oc)]

            NB = len(batches)
            for n in range(NB + DELAY):
                if fin_b_pending:
                    emit_fin_b()
                if n < NB:
                    emit_scores(n)
                if n - DELAY >= 0:
                    emit_consume(n - DELAY)
                if po_pieces:
                    emit_po_piece()
            while fin_b_pending:
                emit_fin_b()
            while po_pieces:
                emit_po_piece()

    nc.compile()
    return nc


def _prep_inputs(hidden_states, attention_mask, cos, sin, Wq, Wk, Wv, Wo, P_list):
    bf16 = ml_dtypes.bfloat16
    hs = np.asarray(hidden_states, dtype=np.float32)
    mask = np.asarray(attention_mask, dtype=np.float32).reshape(T, T)
    cos2 = np.asarray(cos, dtype=np.float32).reshape(T, HD)
    sin2 = np.asarray(sin, dtype=np.float32).reshape(T, HD)
    scale = np.float32(1.0 / np.sqrt(HD))

    def t3(s_):
        # rotate_half add trick: t3 = concat(sin[:, 64:], -sin[:, :64])
        return np.concatenate([s_[:, HD // 2:], -s_[:, :HD // 2]], axis=1)

    wq = np.ascontiguousarray(np.asarray(Wq, dtype=np.float32)).astype(bf16)
    wk = np.ascontiguousarray(np.asarray(Wk, dtype=np.float32)).astype(bf16)
    wv = np.ascontiguousarray(np.asarray(Wv, dtype=np.float32)).astype(bf16)
    wo = np.ascontiguousarray(np.asarray(Wo, dtype=np.float32)).astype(bf16)

    in_maps = []
    for i in range(NC):
        b, pos = i // 4, i % 4
        js = [4 * s + 3 - pos for s in range(NSLOT)]
        take = lambda a: np.ascontiguousarray(
            np.concatenate([a[j * P:(j + 1) * P] for j in js], axis=0))
        xT_rows = take(hs[b])                     # [512, 2048]
        xT = np.ascontiguousarray(xT_rows.T).astype(bf16)
        m_tiles = [
            (mask[js[s] * P:(js[s] + 1) * P,
                  _c_of_j(c) * P:(_c_of_j(c) + 1) * P].T > NEG_THRESH)
            .astype(np.float32)
            for (s, c) in P_list]
        if not m_tiles:
            m_tiles.append(np.ones((P, P), np.float32))
        in_maps.append({
            "xT": xT,
            "wq": wq, "wk": wk, "wv": wv, "wo": wo,
            "cosq": take(cos2 * scale).astype(bf16),
            "sinq3": take(t3(sin2 * scale)).astype(bf16),
            "cosk": take(cos2).astype(bf16),
            "sink3": take(t3(sin2)).astype(bf16),
            "maskbin": np.stack(m_tiles).astype(bf16),
        })
    return in_maps


_cache = {}


def kernel(hidden_states, attention_mask, cos, sin, Wq, Wk, Wv, Wo,
           _trace=False, _trace_kwargs=None):
    from concourse.bass_utils import run_bass_kernel_spmd

    E, P_list = _mask_plan(attention_mask)
    key = (tuple(E), tuple(P_list))
    if key not in _cache:
        _cache[key] = _build_program(E, P_list)
    nc = _cache[key]

    in_maps = _prep_inputs(hidden_states, attention_mask, cos, sin,
                           Wq, Wk, Wv, Wo, P_list)
    kwargs = dict(_trace_kwargs or {})
    if _trace:
        kwargs["trace"] = True
    res = run_bass_kernel_spmd(nc, in_maps, list(range(NC)), **kwargs)

    out = np.empty((B, T, HID), dtype=np.float32)
    for i in range(NC):
        b, pos = i // 4, i % 4
        o = res.results[i]["out"]
        for s in range(NSLOT):
            j = 4 * s + 3 - pos
            out[b, j * P:(j + 1) * P, :] = o[s * P:(s + 1) * P, :]
    kernel._last_result = res
    return out


# revision 23
# speedup vs baseline: 1.2094x; 1.1003x over previous
# Trainium2 Bass kernel for AvaAttention (GQA attention + RoPE + additive mask)
# B=2, T=2048, HID=2048, NH=16, KVH=4, HD=128, fp32 in/out — 8 NeuronCores.
#
# Sharding: sequence-parallel. Core i (batch b=i//4, position p=i%4) owns
# q-blocks j = 4s+3-p of batch b, for slot s in 0..3. Projections are
# row-parallel (weights replicated, host-cast to bf16), K/V exchanged with
# an AllGather over each batch's 4 cores, attention + output projection
# stay local to the core's rows.
#
# v2 design (vs the earlier kernel):
#  - hidden_states arrives pre-transposed from the host (xT), removing the
#    on-device transpose phase entirely.
#  - Attention computes scores directly in [src, (head q)] orientation
#    (stationary = kT block, moving = qT), so the probability tiles come
#    out of exp already transposed for the PV matmul: no per-tile
#    probability transpose, no diag matmuls, no P^T copies on vector.
#  - Softmax denominator Z is accumulated with col-tiled M=1 matmuls
#    (4 concurrent positions at PSUM partitions 0/32/64/96), then one
#    masked-ones matmul sums the partials and broadcasts them across all
#    128 partitions in a single step; a vector reciprocal + one
#    tensor_tensor multiply normalizes ctx while casting to bf16.
#  - The causal mask is applied as a 0/1 multiply on the probability
#    tiles (cheap vector op) instead of -1e9 adds via identity matmuls.
#  - exp runs on the scalar engine in [128, 2048]/[128, 1024] PSUM
#    batches to amortize the per-instruction overhead.
#  - RoPE runs in bf16 on the vector engine (PSUM->bf16 staging copies on
#    the scalar engine) for 2x DVE throughput.

import sys

for _p in ("/opt/trn_rl_repo", "/opt/pypackages"):
    if _p not in sys.path:
        sys.path.insert(0, _p)

import numpy as np
import ml_dtypes

B, T, HID = 2, 2048, 2048
NH, KVH, HD = 16, 4, 128
P = 128
NC = 8
NBLK = T // P          # 16 q-blocks per batch
NSLOT = 4              # blocks per core
GPQ = NH // KVH        # 4 q-heads per kv group
HB = HID // P          # 16 contraction subtiles
NEG_THRESH = -1.0e8


def _c_of_j(j):
    # producer-permuted column-block index (involution)
    return 4 * (j // 4) + 3 - (j % 4)


def _mask_plan(attention_mask):
    """Classify the additive mask per (j, kb) 128x128 tile.

    Returns (E, P_list): E[s] is the uniform extent (in permuted blocks c)
    for slot s; P_list is the ordered list of (s, c) positions where a
    0/1 mask multiply is applied (positions shared by every core; tile
    *data* is per-core).
    """
    m = np.asarray(attention_mask).reshape(T, T)
    nonzero = np.zeros((NBLK, NBLK), dtype=bool)
    live = np.zeros((NBLK, NBLK), dtype=bool)   # not fully masked
    for j in range(NBLK):
        for kb in range(NBLK):
            tile = m[j * P:(j + 1) * P, kb * P:(kb + 1) * P]
            nonzero[j, kb] = bool(np.any(tile != 0.0))
            live[j, kb] = bool(np.any(tile > NEG_THRESH))
    E = []
    for s in range(NSLOT):
        cmax = 1
        for jj in range(4):
            j = 4 * s + jj
            idx = np.nonzero(live[j])[0]
            if len(idx):
                cmax = max(cmax, max(_c_of_j(int(kb)) for kb in idx) + 1)
        E.append(cmax)
    P_list = []
    for s in range(NSLOT):
        for c in range(E[s]):
            kb = _c_of_j(c)
            if any(nonzero[4 * s + jj, kb] for jj in range(4)):
                P_list.append((s, c))
    return E, P_list


def _batches(Es):
    """Split Es blocks into 2-block exp batches alternating tile A / tile B."""
    out = []
    start = 0
    use_a = True
    while start < Es:
        take = min(2, Es - start)
        out.append((start, take, use_a))
        start += take
        use_a = not use_a
    return out


def _build_program(E, P_list):
    import concourse.mybir as mybir
    import concourse.tile as tile
    from concourse import bacc
    from concourse.masks import make_identity
    from contextlib import ExitStack

    FP32 = mybir.dt.float32
    BF16 = mybir.dt.bfloat16
    Exp = mybir.ActivationFunctionType.Exp
    HALF = HD // 2

    nc = bacc.Bacc("TRN2", target_bir_lowering=False, num_devices=NC)

    xT_p = nc.declare_dram_parameter("xT", [HID, NSLOT * P], BF16, isOutput=False)
    wq_p = nc.declare_dram_parameter("wq", [HID, NH * HD], BF16, isOutput=False)
    wk_p = nc.declare_dram_parameter("wk", [HID, KVH * HD], BF16, isOutput=False)
    wv_p = nc.declare_dram_parameter("wv", [HID, KVH * HD], BF16, isOutput=False)
    wo_p = nc.declare_dram_parameter("wo", [HID, HID], BF16, isOutput=False)
    cosq_p = nc.declare_dram_parameter("cosq", [NSLOT * P, HD], BF16, isOutput=False)
    sinq_p = nc.declare_dram_parameter("sinq3", [NSLOT * P, HD], BF16, isOutput=False)
    cosk_p = nc.declare_dram_parameter("cosk", [NSLOT * P, HD], BF16, isOutput=False)
    sink_p = nc.declare_dram_parameter("sink3", [NSLOT * P, HD], BF16, isOutput=False)
    nmask = max(1, len(P_list))
    masks_p = nc.declare_dram_parameter("maskbin", [nmask, P, P], BF16,
                                        isOutput=False)
    out_p = nc.declare_dram_parameter("out", [NSLOT * P, HID], FP32, isOutput=True)

    KVW = KVH * HD  # 512
    ag_k_in = nc.dram_tensor("ag_k_in", [KVW, NSLOT * P], BF16)
    ag_k_out = nc.dram_tensor("ag_k_out", [4, KVW, NSLOT * P], BF16,
                              addr_space="Local")
    ag_v_in = nc.dram_tensor("ag_v_in", [NSLOT * P, KVW], BF16)
    ag_v_out = nc.dram_tensor("ag_v_out", [4, NSLOT * P, KVW], BF16,
                              addr_space="Local")
    groups = [[0, 1, 2, 3], [4, 5, 6, 7]]

    mask_idx = {sk: idx for idx, sk in enumerate(P_list)}

    with tile.TileContext(nc) as tc, ExitStack() as top:
        const = top.enter_context(tc.tile_pool(name="const", bufs=1))
        ident_bf = const.tile([P, P], BF16)
        make_identity(nc, ident_bf[:])
        ones1 = const.tile([P, 1], BF16)
        nc.vector.memset(ones1[:], 1.0)
        ones97 = const.tile([97, P], BF16)
        nc.vector.memset(ones97[:], 0.0)
        for r in (0, 32, 64, 96):
            nc.vector.memset(ones97[r:r + 1, :], 1.0)
        maskbin = const.tile([P, nmask, P], BF16)

        # long-lived attention inputs.  kT/v_all live in SBUF that is
        # disjoint from the projection-phase pools so the gather DMAs can
        # run as soon as each AllGather completes, hidden under the Q
        # projection (a shared region would chain them behind the last
        # projection-phase reader).
        qT_pool = top.enter_context(tc.tile_pool(name="qT_pool", bufs=1))
        qT = qT_pool.tile([P, KVH, NSLOT, GPQ, P], BF16)   # [d, g, s, h, t]
        kv_pool = top.enter_context(tc.tile_pool(name="kv_pool", bufs=1))
        # c = 4*s_src + pos is addressed as [s_src, pos]
        kT = kv_pool.tile([P, KVH, NSLOT, 4, P], BF16)     # [d, g, s_src, pos, t]
        v_all = kv_pool.tile([P, NBLK, KVW], BF16)         # [t%128, c, (g d)]
        ctxT = kv_pool.tile([P, NSLOT, NH, P], BF16)       # [d, s, hh, t]

        # ================= projection phase =================
        with tc.tile_pool(name="xw_pool", bufs=1) as xw_pool, \
             tc.tile_pool(name="qw", bufs=2) as qw_pool, \
             tc.tile_pool(name="stage", bufs=2) as stage, \
             tc.tile_pool(name="ropec", bufs=1) as ropec, \
             tc.tile_pool(name="pps", bufs=1, space="PSUM") as pps:
            xT = xw_pool.tile([P, HB, NSLOT * P], BF16)    # [h%128, hb, t]
            wk_sb = xw_pool.tile([P, HB, KVW], BF16)
            wv_sb = xw_pool.tile([P, HB, KVW], BF16)
            # interleave xT/wk chunk DMAs so the first K-proj matmuls can
            # start after ~128KB instead of after every input tensor
            xTv = xT_p[:].rearrange("(hb p) t -> p hb t", p=P)
            wkv = wk_p[:].rearrange("(hb p) n -> p hb n", p=P)
            for hq in range(4):
                sl = slice(4 * hq, 4 * (hq + 1))
                nc.sync.dma_start(xT[:, sl, :], xTv[:, sl, :])
                nc.sync.dma_start(wk_sb[:, sl, :], wkv[:, sl, :])
            nc.sync.dma_start(wv_sb[:], wv_p[:].rearrange("(hb p) n -> p hb n", p=P))
            cosq_t = ropec.tile([P, NSLOT, HD], BF16)
            sinq_t = ropec.tile([P, NSLOT, HD], BF16)
            cosk_t = ropec.tile([P, NSLOT, HD], BF16)
            sink_t = ropec.tile([P, NSLOT, HD], BF16)
            for ap, prm in ((cosk_t, cosk_p), (sink_t, sink_p),
                            (cosq_t, cosq_p), (sinq_t, sinq_p)):
                nc.sync.dma_start(ap[:], prm[:].rearrange("(s p) d -> p s d", p=P))
            nc.sync.dma_start(maskbin[:], masks_p[:].rearrange("n p d -> p n d"))
            # prefetch the first two Wq chunks
            wq_tiles = {}
            for hc in range(2):
                w = qw_pool.tile([P, HB, GPQ * HD], BF16, name="wq_sb", tag="wq")
                nc.sync.dma_start(
                    w[:], wq_p[:, hc * GPQ * HD:(hc + 1) * GPQ * HD]
                    .rearrange("(hb p) n -> p hb n", p=P))
                wq_tiles[hc] = w

            def rope(dst, src_sb, cos_t, sin_t, s, nh):
                src3 = src_sb[:].rearrange("p (h d) -> p h d", d=HD)
                cst = stage.tile([P, nh, HD], BF16, name="rope_c", tag="rope_c")
                nc.vector.tensor_tensor(
                    dst[:], src3,
                    cos_t[:, s, None, :].to_broadcast((P, nh, HD)),
                    mybir.AluOpType.mult)
                nc.vector.tensor_tensor(
                    cst[:], src3,
                    sin_t[:, s, None, :].to_broadcast((P, nh, HD)),
                    mybir.AluOpType.mult)
                nc.vector.tensor_tensor(dst[:, :, HALF:], dst[:, :, HALF:],
                                        cst[:, :, :HALF], mybir.AluOpType.add)
                nc.vector.tensor_tensor(dst[:, :, :HALF], dst[:, :, :HALF],
                                        cst[:, :, HALF:], mybir.AluOpType.add)

            # ---- K projection + rope + transpose + AllGather ----
            pk = [pps.tile([P, KVW], FP32, name=f"pk{s}", tag=f"pkv{s}")
                  for s in range(NSLOT)]
            for hb in range(HB):
                for s in range(NSLOT):
                    nc.tensor.matmul(pk[s][:], xT[:, hb, s * P:(s + 1) * P],
                                     wk_sb[:, hb, :],
                                     start=(hb == 0), stop=(hb == HB - 1))
            k_rope = []
            for s in range(NSLOT):
                kst = stage.tile([P, KVW], BF16, name=f"kst{s}", tag=f"kst{s % 2}")
                nc.scalar.copy(kst[:], pk[s][:])
                kr = xw_pool.tile([P, KVH, HD], BF16, name=f"k_rope{s}")
                rope(kr, kst, cosk_t, sink_t, s, KVH)
                k_rope.append(kr)
            contrib_k = xw_pool.tile([P, KVH, NSLOT * P], BF16, name="contrib_k")
            for g in range(KVH):
                pkt = pps.tile([P, NSLOT * P], BF16, name="pkt", tag=f"pkt{g % 2}")
                for s in range(NSLOT):
                    nc.tensor.transpose(pkt[:, s * P:(s + 1) * P],
                                        k_rope[s][:, g, :], ident_bf[:])
                nc.vector.tensor_copy(contrib_k[:, g, :], pkt[:])
            nc.sync.dma_start(
                ag_k_in[:].rearrange("(g d) t -> d g t", d=P), contrib_k[:])
            nc.gpsimd.collective_compute(
                "AllGather", mybir.AluOpType.bypass, replica_groups=groups,
                ins=[ag_k_in[:]], outs=[ag_k_out[:]])

            # ---- V projection + AllGather ----
            pv = [pps.tile([P, KVW], FP32, name=f"pv{s}", tag=f"pkv{s}")
                  for s in range(NSLOT)]
            for hb in range(HB):
                for s in range(NSLOT):
                    nc.tensor.matmul(pv[s][:], xT[:, hb, s * P:(s + 1) * P],
                                     wv_sb[:, hb, :],
                                     start=(hb == 0), stop=(hb == HB - 1))
            for s in range(NSLOT):
                vst = stage.tile([P, KVW], BF16, name=f"vst{s}", tag="vst")
                nc.scalar.copy(vst[:], pv[s][:])
                nc.sync.dma_start(ag_v_in[s * P:(s + 1) * P, :], vst[:])
            nc.gpsimd.collective_compute(
                "AllGather", mybir.AluOpType.bypass, replica_groups=groups,
                ins=[ag_v_in[:]], outs=[ag_v_out[:]])

            # ---- Q projection + rope + transpose to qT ----
            for hc in range(NH // GPQ):
                if hc in wq_tiles:
                    wq_sb = wq_tiles[hc]
                else:
                    wq_sb = qw_pool.tile([P, HB, GPQ * HD], BF16,
                                         name="wq_sb", tag="wq")
                    nc.sync.dma_start(
                        wq_sb[:], wq_p[:, hc * GPQ * HD:(hc + 1) * GPQ * HD]
                        .rearrange("(hb p) n -> p hb n", p=P))
                pq = [pps.tile([P, GPQ * HD], FP32, name=f"pq{s}", tag=f"pkv{s}")
                      for s in range(NSLOT)]
                for hb in range(HB):
                    for s in range(NSLOT):
                        nc.tensor.matmul(pq[s][:], xT[:, hb, s * P:(s + 1) * P],
                                         wq_sb[:, hb, :],
                                         start=(hb == 0), stop=(hb == HB - 1))
                for s in range(NSLOT):
                    qst = stage.tile([P, GPQ * HD], BF16, name="qst",
                                     tag=f"qst{s % 2}")
                    nc.scalar.copy(qst[:], pq[s][:])
                    qr = stage.tile([P, GPQ, HD], BF16, name="q_rope",
                                    tag=f"q_rope{s % 2}")
                    rope(qr, qst, cosq_t, sinq_t, s, GPQ)
                    pqt = pps.tile([P, GPQ * P], BF16, name="pqt",
                                   tag=f"pqt{s % 2}")
                    for h in range(GPQ):
                        nc.tensor.transpose(pqt[:, h * P:(h + 1) * P],
                                            qr[:, h, :], ident_bf[:])
                    nc.vector.tensor_copy(
                        qT[:, hc, s, :, :],
                        pqt[:].rearrange("p (h t) -> p h t", t=P))

            # ---- gather K/V from the AllGather outputs ----
            # interleaved so the first attention group's inputs arrive first
            for i in range(4):
                for pos in range(4):
                    nc.sync.dma_start(
                        kT[:, i, :, pos, :],
                        ag_k_out[pos, i * P:(i + 1) * P, :]
                        .rearrange("d (s t) -> d s t", t=P))
                for pos in range(4):
                    nc.sync.dma_start(v_all[:, 4 * i + pos, :],
                                      ag_v_out[pos, i * P:(i + 1) * P, :])

        # wo preload overlaps attention (SBUF freed by the projection pools);
        # chunked by output column so the first out-proj chain can start early
        wo_sb = top.enter_context(tc.tile_pool(name="wo_pool", bufs=1)) \
            .tile([P, HB, HID], BF16)
        wov = wo_p[:].rearrange("(hb p) n -> p hb n", p=P)
        for oc in range(HID // 512):
            nc.sync.dma_start(wo_sb[:, :, oc * 512:(oc + 1) * 512],
                              wov[:, :, oc * 512:(oc + 1) * 512])

        # ============ attention + interleaved output projection ============
        # Flat software pipeline over all (slot, group, batch) score batches:
        # the Z/PV consumption of batch n-2 is emitted together with the
        # scores of batch n, so the in-order tensor queue never waits on the
        # scalar engine's exp.  Output-projection chains are split into
        # 4-matmul pieces injected one per pipeline step; the softmax
        # finalize (Z sum/broadcast, reciprocal, normalize) of each group is
        # staged across two steps.  The goal is a gapless tensor stream:
        # any multi-us PE idle drops the PE array to its half-pump state.
        with tc.tile_pool(name="pt_pool", bufs=2) as pt_pool, \
             tc.tile_pool(name="astage", bufs=2) as astage, \
             tc.tile_pool(name="ostage", bufs=2) as ostage, \
             tc.tile_pool(name="aps", bufs=1, space="PSUM") as aps, \
             tc.tile_pool(name="cps", bufs=2, space="PSUM") as cps:

            DELAY = 2
            batches = []      # (s, g, c0, blk, tag, is_first, is_last)
            for s in range(NSLOT):
                for g in range(KVH):
                    bl = _batches(E[s])
                    for bi, (c0, blk, use_a) in enumerate(bl):
                        batches.append((s, g, c0, blk,
                                        "scA" if use_a else "scB",
                                        bi == 0, bi == len(bl) - 1))

            sc_tiles = {}     # stream index -> score psum tile
            group_state = {}  # (s, g) -> dict(pT=, zp=, pctx=)
            po_pieces = []    # queue of (s, oc, piece_idx)
            po_live = {}      # (s, oc) -> psum tile
            fin_b_pending = []

            def emit_scores(n):
                s, g, c0, blk, tag, first, last = batches[n]
                if first:
                    st = group_state[(s, g)] = {}
                    st["pT"] = pt_pool.tile([P, E[s], GPQ * P], BF16,
                                            name="pT", tag="pT")
                sc = aps.tile([P, 1024], FP32, name="sc", tag=tag)
                sc_tiles[n] = sc
                st = group_state[(s, g)]
                scv = sc[:].rearrange("p (b n) -> p b n", n=512)
                qmov = qT[:, g, s, :, :]
                for i in range(blk):
                    c = c0 + i
                    nc.tensor.matmul(scv[:, i, :],
                                     kT[:, g, c // 4, c % 4, :], qmov,
                                     start=True, stop=True)
                pT = st["pT"]
                nc.scalar.activation(pT[:, c0:c0 + blk, :], scv[:, :blk, :], Exp)
                for i in range(blk):
                    c = c0 + i
                    mi = mask_idx.get((s, c))
                    if mi is not None:
                        pslice = pT[:, c, :].rearrange("p (h t) -> p h t", t=P)
                        nc.vector.tensor_tensor(
                            pslice, pslice,
                            maskbin[:, mi, None, :].to_broadcast((P, GPQ, P)),
                            mybir.AluOpType.mult)

            def emit_consume(n):
                s, g, c0, blk, tag, first, last = batches[n]
                Es = E[s]
                st = group_state[(s, g)]
                if first:
                    zp = st["zp"] = aps.tile([P, 512], FP32, name="zp", tag="zp")
                    if (s, g) == (0, 0):
                        # clear stale PSUM once in the rows ones97 reads but
                        # the Z chains never write (0 * NaN would poison the
                        # sum); the zp ring owns this bank, and the unwritten
                        # rows stay zero for the rest of the kernel
                        nc.vector.memset(zp[0:97, :], 0.0)
                    st["pctx"] = cps.tile([P, GPQ * P], FP32,
                                          name="pctx", tag="pctx")
                pT, zp, pctx = st["pT"], st["zp"], st["pctx"]
                for i in range(blk):
                    c = c0 + i
                    pos = c % 4
                    nc.tensor.matmul(
                        zp[32 * pos:32 * pos + 1, :], ones1[:], pT[:, c, :],
                        start=(c < 4), stop=(c + 4 >= Es),
                        tile_position=(0, 32 * pos))
                for i in range(blk):
                    c = c0 + i
                    nc.tensor.matmul(
                        pctx[:], v_all[:, c, g * HD:(g + 1) * HD], pT[:, c, :],
                        start=(c == 0), stop=(c == Es - 1))
                if last:
                    zsb = astage.tile([97, 512], BF16, name="zsb", tag="zsb")
                    nc.vector.tensor_copy(zsb[:], zp[0:97, :])
                    fin_b_pending.append((s, g, zsb, pctx))

            def emit_fin_b():
                s, g, zsb, pctx = fin_b_pending.pop(0)
                zb = aps.tile([P, 1024], FP32, name="zb", tag="scB")
                nc.tensor.matmul(zb[:, :512], ones97[:], zsb[:],
                                 start=True, stop=True)
                rz = astage.tile([P, 512], FP32, name="rz", tag="rz")
                nc.vector.reciprocal_approx_fast(rz[:], zb[:, :512])
                nc.vector.tensor_tensor(
                    ctxT[:, s, g * GPQ:(g + 1) * GPQ, :]
                    .rearrange("p h t -> p (h t)"),
                    pctx[:], rz[:], mybir.AluOpType.mult)
                if g == KVH - 1:
                    po_pieces.extend((s, oc, pc)
                                     for oc in range(HID // 512)
                                     for pc in range(4))

            def emit_po_piece():
                s, oc, pc = po_pieces.pop(0)
                if pc == 0:
                    po_live[(s, oc)] = aps.tile([P, 512], FP32,
                                                name="po", tag="po")
                po = po_live[(s, oc)]
                for hh in range(4 * pc, 4 * pc + 4):
                    nc.tensor.matmul(po[:], ctxT[:, s, hh, :],
                                     wo_sb[:, hh, oc * 512:(oc + 1) * 512],
                                     start=(hh == 0), stop=(hh == NH - 1),
                                     skip_group_check=True)
                if pc == 3:
                    ot = ostage.tile([P, 512], FP32, name="ot", tag="ot")
                    nc.vector.tensor_copy(ot[:], po[:])
                    nc.sync.dma_start(
                        out_p[s * P:(s + 1) * P, oc * 512:(oc + 1) * 512],
                        ot[:])
                    del po_live[(s, oc)]

            NB = len(batches)
            for n in range(NB + DELAY):
                if fin_b_pending:
                    emit_fin_b()
                if n < NB:
                    emit_scores(n)
                if n - DELAY >= 0:
                    emit_consume(n - DELAY)
                if po_pieces:
                    emit_po_piece()
            while fin_b_pending:
                emit_fin_b()
            while po_pieces:
                emit_po_piece()

    nc.compile()
    return nc


def _prep_inputs(hidden_states, attention_mask, cos, sin, Wq, Wk, Wv, Wo, P_list):
    bf16 = ml_dtypes.bfloat16
    hs = np.asarray(hidden_states, dtype=np.float32)
    mask = np.asarray(attention_mask, dtype=np.float32).reshape(T, T)
    cos2 = np.asarray(cos, dtype=np.float32).reshape(T, HD)
    sin2 = np.asarray(sin, dtype=np.float32).reshape(T, HD)
    scale = np.float32(1.0 / np.sqrt(HD))

    def t3(s_):
        # rotate_half add trick: t3 = concat(sin[:, 64:], -sin[:, :64])
        return np.concatenate([s_[:, HD // 2:], -s_[:, :HD // 2]], axis=1)

    wq = np.ascontiguousarray(np.asarray(Wq, dtype=np.float32)).astype(bf16)
    wk = np.ascontiguousarray(np.asarray(Wk, dtype=np.float32)).astype(bf16)
    wv = np.ascontiguousarray(np.asarray(Wv, dtype=np.float32)).astype(bf16)
    wo = np.ascontiguousarray(np.asarray(Wo, dtype=np.float32)).astype(bf16)

    in_maps = []
    for i in range(NC):
        b, pos = i // 4, i % 4
        js = [4 * s + 3 - pos for s in range(NSLOT)]
        take = lambda a: np.ascontiguousarray(
            np.concatenate([a[j * P:(j + 1) * P] for j in js], axis=0))
        xT_rows = take(hs[b])                     # [512, 2048]
        xT = np.ascontiguousarray(xT_rows.T).astype(bf16)
        m_tiles = [
            (mask[js[s] * P:(js[s] + 1) * P,
                  _c_of_j(c) * P:(_c_of_j(c) + 1) * P].T > NEG_THRESH)
            .astype(np.float32)
            for (s, c) in P_list]
        if not m_tiles:
            m_tiles.append(np.ones((P, P), np.float32))
        in_maps.append({
            "xT": xT,
            "wq": wq, "wk": wk, "wv": wv, "wo": wo,
            "cosq": take(cos2 * scale).astype(bf16),
            "sinq3": take(t3(sin2 * scale)).astype(bf16),
            "cosk": take(cos2).astype(bf16),
            "sink3": take(t3(sin2)).astype(bf16),
            "maskbin": np.stack(m_tiles).astype(bf16),
        })
    return in_maps


_cache = {}


def kernel(hidden_states, attention_mask, cos, sin, Wq, Wk, Wv, Wo,
           _trace=False, _trace_kwargs=None):
    from concourse.bass_utils import run_bass_kernel_spmd

    E, P_list = _mask_plan(attention_mask)
    key = (tuple(E), tuple(P_list))
    if key not in _cache:
        _cache[key] = _build_program(E, P_list)
    nc = _cache[key]

    in_maps = _prep_inputs(hidden_states, attention_mask, cos, sin,
                           Wq, Wk, Wv, Wo, P_list)
    kwargs = dict(_trace_kwargs or {})
    if _trace:
        kwargs["trace"] = True
    res = run_bass_kernel_spmd(nc, in_maps, list(range(NC)), **kwargs)

    out = np.empty((B, T, HID), dtype=np.float32)
    for i in range(NC):
        b, pos = i // 4, i % 4
        o = res.results[i]["out"]
        for s in range(NSLOT):
            j = 4 * s + 3 - pos
            out[b, j * P:(j + 1) * P, :] = o[s * P:(s + 1) * P, :]
    kernel._last_result = res
    return out
